# revision 5
# baseline (speedup 1.0000x reference)
"""Causal MHA (B=4, S=2048, D=1024, H=16) on 8 NeuronCores — v3.

v2 + fp8-DoubleRow QK projection, reciprocal_approx_fast normalization,
DMA ordering, and software-pipelined QK/proj steps injected into the
attention kc loop so the PE stays busy while ACT does the exps.
"""
import sys

for _p in ("/opt/trn_rl_repo", "/root/.axon_site/_ro/trn_rl_repo"):
    if _p not in sys.path:
        sys.path.append(_p)

import numpy as np
import ml_dtypes

import concourse.bass as bass
import concourse.mybir as mybir
import concourse.tile as tile
from concourse import bacc

B, S, D, H = 4, 2048, 1024, 16
HD = D // H            # 64
NHL = 8                # heads per core
QB = 1024              # attention q-block
NKC = S // 128         # 16 k-chunks
dt = mybir.dt
AF = mybir.ActivationFunctionType
DR = mybir.MatmulPerfMode.DoubleRow
P = 128
BF = ml_dtypes.bfloat16
QK_FP8 = False
W8SCALE = 32.0         # host pre-scale on Wq/Wk for fp8 range
EXP_SCALE = 0.125 / (W8SCALE * W8SCALE) if QK_FP8 else 0.125


def build_nc(repeat=1):
    nc = bacc.Bacc("TRN2", target_bir_lowering=False, debug=False)

    xt = nc.dram_tensor("xt", [P, 8, S], dt.bfloat16, kind="ExternalInput")
    if QK_FP8:
        xt8 = nc.dram_tensor("xt8", [P, 8, S], dt.float8e4, kind="ExternalInput")
        wqk8 = nc.dram_tensor("wqk8", [P, 4, 2, 8, P], dt.float8e4, kind="ExternalInput")
    else:
        wqk = nc.dram_tensor("wqk", [P, 8, 8, P], dt.bfloat16, kind="ExternalInput")
    wv = nc.dram_tensor("wv", [P, 8, 512], dt.bfloat16, kind="ExternalInput")
    wpj = nc.dram_tensor("wpj", [P, 4, D], dt.bfloat16, kind="ExternalInput")
    out = nc.dram_tensor("out", [S, D], dt.bfloat16, kind="ExternalOutput")

    from contextlib import ExitStack
    with tile.TileContext(nc) as tc, ExitStack() as _rep:
        if repeat > 1:
            _rep.enter_context(tc.For_i(0, repeat, 1))
        with tc.tile_pool(name="persist", bufs=1) as pp:
            QT = pp.tile([P, 4, S], dt.bfloat16, tag="QT")
            KT = pp.tile([P, 4, S], dt.bfloat16, tag="KT")
            V2 = pp.tile([P, NHL, NKC, P], dt.bfloat16, tag="V2")
            nc.gpsimd.memset(V2[:, 0:NHL:2, :, 64:P], 1.0)
            nc.gpsimd.memset(V2[:, 1:NHL:2, :, 0:64], 1.0)
            yT = pp.tile([P, 4, S], dt.bfloat16, tag="yT")
            xTt = pp.tile([P, 8, S], dt.bfloat16, tag="xTt")
            wvT = pp.tile([P, 8, 512], dt.bfloat16, tag="wvT")
            wpjT = pp.tile([P, 4, D], dt.bfloat16, tag="wpjT")
            if QK_FP8:
                x8T = pp.tile([P, 8, S], dt.float8e4, tag="x8T")
                wqk8T = pp.tile([P, 4, 2, 8, P], dt.float8e4, tag="wqk8T")
            else:
                wqkT = pp.tile([P, 8, 8, P], dt.bfloat16, tag="wqkT")

            # DMA order: V-phase inputs first, QK weights next, proj last
            nc.sync.dma_start(wvT[:, 0:4, :], wv[:, 0:4, :])
            nc.sync.dma_start(wvT[:, 4:8, :], wv[:, 4:8, :])
            nc.sync.dma_start(xTt[:, :, 0:P], xt[:, :, 0:P])
            nc.sync.dma_start(xTt[:, :, P:512], xt[:, :, P:512])
            for sb in range(1, 4):
                nc.sync.dma_start(xTt[:, :, sb * 512:(sb + 1) * 512],
                                  xt[:, :, sb * 512:(sb + 1) * 512])
            if QK_FP8:
                nc.sync.dma_start(wqk8T[:], wqk8[:])
                for sb in range(4):
                    nc.sync.dma_start(x8T[:, :, sb * 512:(sb + 1) * 512],
                                      xt8[:, :, sb * 512:(sb + 1) * 512])
            else:
                nc.sync.dma_start(wqkT[:], wqk[:])
            nc.sync.dma_start(wpjT[:], wpj[:])

            # ---------------- Phase V: V natural [s, hd] ----------------
            with tc.tile_pool(name="psV0", bufs=4, space="PSUM") as psV0:
                for sc in range(16):
                    psv = psV0.tile([P, 512], dt.float32, tag="psv")
                    for dc in range(8):
                        nc.tensor.matmul(psv[:], xTt[:, dc, sc * P:(sc + 1) * P],
                                         wvT[:, dc, :], start=(dc == 0), stop=(dc == 7))
                    pv = psv[:].rearrange("p (h e) -> p h e", h=8)
                    nc.vector.tensor_copy(V2[:, 0:NHL:2, sc, 0:64], pv[:, 0:8:2, :])
                    nc.vector.tensor_copy(V2[:, 1:NHL:2, sc, 64:P], pv[:, 1:8:2, :])

            # ------------- main pipeline: QK + attention + proj -------------
            with tc.tile_pool(name="ta", bufs=2) as ta, \
                 tc.tile_pool(name="tpt", bufs=6) as tpt, \
                 tc.tile_pool(name="tso", bufs=4) as tso, \
                 tc.tile_pool(name="psS", bufs=2, space="PSUM") as psS, \
                 tc.tile_pool(name="psPV", bufs=1, space="PSUM") as psPV, \
                 tc.tile_pool(name="psQ", bufs=1, space="PSUM") as psQ:

                def qk_steps(pr):
                    """Yield fine-grained steps computing QT/KT for pair pr."""
                    for ch in (pr, pr + 4):
                        dst = QT if ch < 4 else KT
                        for hp in range(2):          # sb pairs (0,1) and (2,3)
                            psq = psQ.tile([P, 2, 512], dt.float32, tag="psq")
                            if QK_FP8:
                                for dcp in range(4):
                                    for sbi in range(2):
                                        sb = 2 * hp + sbi
                                        nc.tensor.matmul(
                                            psq[:, sbi, :],
                                            wqk8T[:, dcp, :, ch, :],
                                            x8T[:, 2 * dcp:2 * dcp + 2,
                                                sb * 512:(sb + 1) * 512],
                                            start=(dcp == 0), stop=(dcp == 3),
                                            perf_mode=DR)
                                    yield
                            else:
                                for dc in range(8):
                                    for sbi in range(2):
                                        sb = 2 * hp + sbi
                                        nc.tensor.matmul(
                                            psq[:, sbi, :],
                                            wqkT[:, dc, ch, :],
                                            xTt[:, dc, sb * 512:(sb + 1) * 512],
                                            start=(dc == 0), stop=(dc == 7))
                                    yield
                            nc.vector.tensor_copy(
                                dst[:, ch % 4, 2 * hp * 512:(2 * hp + 2) * 512],
                                psq[:].rearrange("p a b -> p (a b)"))
                            yield

                def proj_steps(qb):
                    """Yield steps for the output projection of q-block qb."""
                    for sc in range(qb * 8, (qb + 1) * 8):
                        pps = psQ.tile([P, 2, 512], dt.float32, tag="psq")
                        for oc in range(2):
                            for pc in range(4):
                                nc.tensor.matmul(pps[:, oc, :],
                                                 yT[:, pc, sc * P:(sc + 1) * P],
                                                 wpjT[:, pc, oc * 512:(oc + 1) * 512],
                                                 start=(pc == 0), stop=(pc == 3))
                            yield
                        so = tso.tile([P, 2, 512], dt.bfloat16, tag="so")
                        nc.vector.tensor_copy(so[:], pps[:])
                        nc.sync.dma_start(
                            out[sc * P:(sc + 1) * P, :],
                            so[:].rearrange("p a b -> p (a b)"))
                        yield

                def attn(h, qb, filler):
                    pr = h // 2
                    half = slice(0, 64) if h % 2 == 0 else slice(64, P)
                    ysl = slice(0, 64) if h % 2 == 0 else slice(64, P)
                    ssl = slice(64, P) if h % 2 == 0 else slice(0, 64)
                    nkc = (qb + 1) * 8
                    pv_ps = psPV.tile([P, QB], dt.float32, tag="pv")
                    pend = []          # up to 2 deferred PV emissions

                    def emit_pv(kc, pT_t, qlo):
                        q0 = qlo
                        while q0 < QB:
                            q1 = min((q0 // 512 + 1) * 512, QB)
                            nc.tensor.matmul(pv_ps[:, q0:q1],
                                             V2[:, h, kc, :], pT_t[:, q0:q1],
                                             start=(kc == 0), stop=(kc == nkc - 1),
                                             skip_group_check=True)
                            q0 = q1

                    for kc in range(nkc):
                        qlo = max(0, kc * P - qb * QB)
                        sc_ps = psS.tile([P, QB], dt.float32, tag="sc")
                        q0 = qlo
                        while q0 < QB:
                            q1 = min((q0 // 512 + 1) * 512, QB)
                            nc.tensor.matmul(sc_ps[:, q0:q1],
                                             KT[half, pr, kc * P:(kc + 1) * P],
                                             QT[half, pr, qb * QB + q0:qb * QB + q1],
                                             start=True, stop=True)
                            q0 = q1
                        pT_t = tpt.tile([P, QB], dt.bfloat16, tag="pT")
                        nc.scalar.activation(pT_t[:, qlo:QB], sc_ps[:, qlo:QB],
                                             AF.Exp, scale=EXP_SCALE)
                        if kc * P >= qb * QB:
                            nc.gpsimd.affine_select(
                                out=pT_t[:, qlo:qlo + P], in_=pT_t[:, qlo:qlo + P],
                                compare_op=mybir.AluOpType.is_ge, fill=0.0,
                                base=0, pattern=[[1, P]], channel_multiplier=-1)
                        if len(pend) >= 2:
                            emit_pv(*pend.pop(0))
                        pend.append((kc, pT_t, qlo))
                        if filler:
                            try:
                                next(filler)
                            except StopIteration:
                                filler = None
                    for p_ in pend:
                        emit_pv(*p_)

                    # sums -> SBUF base-0 (approx recip is only exact there),
                    # then scale y rows with a mixed-space cross-partition mult
                    ssum = ta.tile([64, QB], dt.float32, tag="ssum")
                    nc.vector.tensor_copy(ssum[:], pv_ps[ssl, :])
                    rr = ta.tile([64, QB], dt.float32, tag="rr")
                    nc.vector.reciprocal_approx_fast(rr[:], ssum[:])
                    nc.vector.tensor_tensor(yT[ysl, pr, qb * QB:(qb + 1) * QB],
                                            pv_ps[ysl, :], rr[:],
                                            mybir.AluOpType.mult)
                    return filler

                # prologue: QK for pair 0, not overlapped
                for _ in qk_steps(0):
                    pass
                for pr in range(4):
                    filler = qk_steps(pr + 1) if pr < 3 else None
                    for h in (2 * pr, 2 * pr + 1):
                        for qb in range(2):
                            if pr == 3 and h == 7 and qb == 1:
                                filler = proj_steps(0)
                            filler = attn(h, qb, filler)
                    # drain any leftover filler steps
                    while filler is not None:
                        try:
                            next(filler)
                        except StopIteration:
                            filler = None

            # tail: projection of q-block 1
            with tc.tile_pool(name="tso2", bufs=4) as tso2, \
                 tc.tile_pool(name="psP", bufs=4, space="PSUM") as psP:
                for sc in range(8, 16):
                    for oc in range(2):
                        pps = psP.tile([P, 512], dt.float32, tag="pp")
                        for pc in range(4):
                            nc.tensor.matmul(pps[:], yT[:, pc, sc * P:(sc + 1) * P],
                                             wpjT[:, pc, oc * 512:(oc + 1) * 512],
                                             start=(pc == 0), stop=(pc == 3))
                        so = tso2.tile([P, 512], dt.bfloat16, tag="so2")
                        nc.vector.tensor_copy(so[:], pps[:])
                        nc.sync.dma_start(
                            out[sc * P:(sc + 1) * P, oc * 512:(oc + 1) * 512], so[:])

    nc.compile()
    return nc


def prepare_inputs(x, Wqkv, Wproj):
    """Pack per-core inputs. Core c: batch c//2, heads (c%2)*8 .. +8."""
    x = np.asarray(x, dtype=np.float32)
    Wqkv = np.asarray(Wqkv, dtype=np.float32)
    Wproj = np.asarray(Wproj, dtype=np.float32)
    F8 = mybir.dt.np(dt.float8e4)
    Wq = Wqkv[:, :D].reshape(8, P, H, HD)        # [dc, p, head, hd]
    Wk = Wqkv[:, D:2 * D].reshape(8, P, H, HD)
    Wv_ = Wqkv[:, 2 * D:].reshape(8, P, H, HD)
    in_maps = []
    for c in range(8):
        b, g = c // 2, c % 2
        hg = g * NHL
        xtr = np.ascontiguousarray(x[b].T.reshape(8, P, S).transpose(1, 0, 2))
        wqk = np.empty((P, 8, 8, P), dtype=np.float32)
        for ch in range(4):
            wqk[:, :, ch, 0:64] = Wq[:, :, hg + 2 * ch, :].transpose(1, 0, 2)
            wqk[:, :, ch, 64:P] = Wq[:, :, hg + 2 * ch + 1, :].transpose(1, 0, 2)
            wqk[:, :, ch + 4, 0:64] = Wk[:, :, hg + 2 * ch, :].transpose(1, 0, 2)
            wqk[:, :, ch + 4, 64:P] = Wk[:, :, hg + 2 * ch + 1, :].transpose(1, 0, 2)
        wv = Wv_[:, :, hg:hg + NHL, :].reshape(8, P, NHL * HD).transpose(1, 0, 2)
        wpj = np.empty((P, 4, D), dtype=np.float32)
        for pc in range(4):
            wpj[0:64, pc, :] = Wproj[HD * (hg + 2 * pc):HD * (hg + 2 * pc) + HD, :]
            wpj[64:P, pc, :] = Wproj[HD * (hg + 2 * pc + 1):HD * (hg + 2 * pc + 1) + HD, :]
        m = {
            "xt": xtr.astype(BF),
            "wv": np.ascontiguousarray(wv).astype(BF),
            "wpj": wpj.astype(BF),
        }
        if QK_FP8:
            m["xt8"] = xtr.astype(F8)
            m["wqk8"] = np.ascontiguousarray(
                (wqk * W8SCALE).reshape(P, 4, 2, 8, P)).astype(F8)
        else:
            m["wqk"] = wqk.astype(BF)
        in_maps.append(m)
    return in_maps


def combine_outputs(results):
    out = np.empty((B, S, D), dtype=np.float32)
    for b in range(B):
        out[b] = results[2 * b]["out"].astype(np.float32) + \
                 results[2 * b + 1]["out"].astype(np.float32)
    return out


_NC_CACHE = None


def get_nc():
    global _NC_CACHE
    if _NC_CACHE is None:
        _NC_CACHE = build_nc()
    return _NC_CACHE


def kernel(x, Wqkv, Wproj):
    from concourse.bass_utils import run_bass_kernel_spmd
    nc = get_nc()
    in_maps = prepare_inputs(x, Wqkv, Wproj)
    res = run_bass_kernel_spmd(nc, in_maps, core_ids=list(range(8)))
    return combine_outputs(res.results)


if __name__ == "__main__":
    rng = np.random.default_rng(0)
    x = rng.standard_normal((B, S, D), dtype=np.float32)
    Wqkv = (rng.standard_normal((D, 3 * D), dtype=np.float32) / np.sqrt(D)).astype(np.float32)
    Wproj = (rng.standard_normal((D, D), dtype=np.float32) / np.sqrt(D)).astype(np.float32)
    y = kernel(x, Wqkv, Wproj)
    print("ok", y.shape, float(np.abs(y).max()))


# revision 6
# speedup vs baseline: 1.0457x; 1.0457x over previous
"""Causal MHA (B=4, S=2048, D=1024, H=16) on 8 Trainium2 NeuronCores.

Sharding: core c handles batch c//2 and head-group c%2 (8 of 16 heads);
the host sums the two half-head output partials per batch.

All-bf16 datapath, zero PE transposes (host pre-transposes x; V computed
in natural [s, hd] layout; Q^T/K^T come out of the projection in
scores-ready layout). Attention row sums ride along in the PV matmul via
ones-columns in the stationary (flipped per head parity); normalization
is reciprocal_approx_fast on an SBUF base-0 staging tile plus a
mixed-space cross-partition multiply. The causal mask is applied
post-exp by zeroing the diagonal block's upper triangle on gpsimd.
QK-projection and output-projection matmuls are software-pipelined into
the attention kc loop as filler steps, and PV emission trails exp by two
k-chunks so the tensor engine never waits on the scalar engine.
"""
import sys

for _p in ("/opt/trn_rl_repo", "/root/.axon_site/_ro/trn_rl_repo"):
    if _p not in sys.path:
        sys.path.append(_p)

import numpy as np
import ml_dtypes

import concourse.bass as bass
import concourse.mybir as mybir
import concourse.tile as tile
from concourse import bacc

B, S, D, H = 4, 2048, 1024, 16
HD = D // H            # 64
NHL = 8                # heads per core
QB = 1024              # attention q-block
NKC = S // 128         # 16 k-chunks
dt = mybir.dt
AF = mybir.ActivationFunctionType
DR = mybir.MatmulPerfMode.DoubleRow
P = 128
BF = ml_dtypes.bfloat16
QK_FP8 = False
W8SCALE = 32.0         # host pre-scale on Wq/Wk for fp8 range
EXP_SCALE = 0.125 / (W8SCALE * W8SCALE) if QK_FP8 else 0.125


def build_nc(repeat=1):
    nc = bacc.Bacc("TRN2", target_bir_lowering=False, debug=False)

    xt = nc.dram_tensor("xt", [P, 8, S], dt.bfloat16, kind="ExternalInput")
    if QK_FP8:
        xt8 = nc.dram_tensor("xt8", [P, 8, S], dt.float8e4, kind="ExternalInput")
        wqk8 = nc.dram_tensor("wqk8", [P, 4, 2, 8, P], dt.float8e4, kind="ExternalInput")
    else:
        wqk = nc.dram_tensor("wqk", [P, 8, 8, P], dt.bfloat16, kind="ExternalInput")
    wv = nc.dram_tensor("wv", [P, 8, 512], dt.bfloat16, kind="ExternalInput")
    wpj = nc.dram_tensor("wpj", [P, 4, D], dt.bfloat16, kind="ExternalInput")
    out = nc.dram_tensor("out", [S, D], dt.bfloat16, kind="ExternalOutput")

    from contextlib import ExitStack
    with tile.TileContext(nc) as tc, ExitStack() as _rep:
        if repeat > 1:
            _rep.enter_context(tc.For_i(0, repeat, 1))
        with tc.tile_pool(name="persist", bufs=1) as pp:
            QT = pp.tile([P, 4, S], dt.bfloat16, tag="QT")
            KT = pp.tile([P, 4, S], dt.bfloat16, tag="KT")
            V2 = pp.tile([P, NHL, NKC, P], dt.bfloat16, tag="V2")
            nc.gpsimd.memset(V2[:, 0:NHL:2, :, 64:P], 1.0)
            nc.gpsimd.memset(V2[:, 1:NHL:2, :, 0:64], 1.0)
            yT = pp.tile([P, 4, S], dt.bfloat16, tag="yT")
            xTt = pp.tile([P, 8, S], dt.bfloat16, tag="xTt")
            wvT = pp.tile([P, 8, 512], dt.bfloat16, tag="wvT")
            wpjT = pp.tile([P, 4, D], dt.bfloat16, tag="wpjT")
            if QK_FP8:
                x8T = pp.tile([P, 8, S], dt.float8e4, tag="x8T")
                wqk8T = pp.tile([P, 4, 2, 8, P], dt.float8e4, tag="wqk8T")
            else:
                wqkT = pp.tile([P, 8, 8, P], dt.bfloat16, tag="wqkT")

            # DMA order: V-phase inputs first, QK weights next, proj last
            nc.sync.dma_start(wvT[:, 0:4, :], wv[:, 0:4, :])
            nc.sync.dma_start(wvT[:, 4:8, :], wv[:, 4:8, :])
            nc.sync.dma_start(xTt[:, :, 0:P], xt[:, :, 0:P])
            nc.sync.dma_start(xTt[:, :, P:512], xt[:, :, P:512])
            for sb in range(1, 4):
                nc.sync.dma_start(xTt[:, :, sb * 512:(sb + 1) * 512],
                                  xt[:, :, sb * 512:(sb + 1) * 512])
            if QK_FP8:
                nc.sync.dma_start(wqk8T[:], wqk8[:])
                for sb in range(4):
                    nc.sync.dma_start(x8T[:, :, sb * 512:(sb + 1) * 512],
                                      xt8[:, :, sb * 512:(sb + 1) * 512])
            else:
                nc.sync.dma_start(wqkT[:], wqk[:])
            nc.sync.dma_start(wpjT[:], wpj[:])

            # ---------------- Phase V: V natural [s, hd] ----------------
            with tc.tile_pool(name="psV0", bufs=4, space="PSUM") as psV0:
                for sc in range(16):
                    psv = psV0.tile([P, 512], dt.float32, tag="psv")
                    for dc in range(8):
                        nc.tensor.matmul(psv[:], xTt[:, dc, sc * P:(sc + 1) * P],
                                         wvT[:, dc, :], start=(dc == 0), stop=(dc == 7))
                    pv = psv[:].rearrange("p (h e) -> p h e", h=8)
                    nc.vector.tensor_copy(V2[:, 0:NHL:2, sc, 0:64], pv[:, 0:8:2, :])
                    nc.vector.tensor_copy(V2[:, 1:NHL:2, sc, 64:P], pv[:, 1:8:2, :])

            # ------------- main pipeline: QK + attention + proj -------------
            with tc.tile_pool(name="ta", bufs=2) as ta, \
                 tc.tile_pool(name="tpt", bufs=6) as tpt, \
                 tc.tile_pool(name="tso", bufs=4) as tso, \
                 tc.tile_pool(name="psS", bufs=2, space="PSUM") as psS, \
                 tc.tile_pool(name="psPV", bufs=1, space="PSUM") as psPV, \
                 tc.tile_pool(name="psQ", bufs=1, space="PSUM") as psQ:

                def qk_steps(pr):
                    """Yield fine-grained steps computing QT/KT for pair pr."""
                    for ch in (pr, pr + 4):
                        dst = QT if ch < 4 else KT
                        for hp in range(2):          # sb pairs (0,1) and (2,3)
                            psq = psQ.tile([P, 2, 512], dt.float32, tag="psq")
                            if QK_FP8:
                                for dcp in range(4):
                                    for sbi in range(2):
                                        sb = 2 * hp + sbi
                                        nc.tensor.matmul(
                                            psq[:, sbi, :],
                                            wqk8T[:, dcp, :, ch, :],
                                            x8T[:, 2 * dcp:2 * dcp + 2,
                                                sb * 512:(sb + 1) * 512],
                                            start=(dcp == 0), stop=(dcp == 3),
                                            perf_mode=DR)
                                    yield
                            else:
                                for dc in range(8):
                                    for sbi in range(2):
                                        sb = 2 * hp + sbi
                                        nc.tensor.matmul(
                                            psq[:, sbi, :],
                                            wqkT[:, dc, ch, :],
                                            xTt[:, dc, sb * 512:(sb + 1) * 512],
                                            start=(dc == 0), stop=(dc == 7))
                                    yield
                            nc.vector.tensor_copy(
                                dst[:, ch % 4, 2 * hp * 512:(2 * hp + 2) * 512],
                                psq[:].rearrange("p a b -> p (a b)"))
                            yield

                def proj_steps(qb):
                    """Yield steps for the output projection of q-block qb."""
                    for sc in range(qb * 8, (qb + 1) * 8):
                        pps = psQ.tile([P, 2, 512], dt.float32, tag="psq")
                        for oc in range(2):
                            for pc in range(4):
                                nc.tensor.matmul(pps[:, oc, :],
                                                 yT[:, pc, sc * P:(sc + 1) * P],
                                                 wpjT[:, pc, oc * 512:(oc + 1) * 512],
                                                 start=(pc == 0), stop=(pc == 3))
                            yield
                        so = tso.tile([P, 2, 512], dt.bfloat16, tag="so")
                        nc.vector.tensor_copy(so[:], pps[:])
                        nc.sync.dma_start(
                            out[sc * P:(sc + 1) * P, :],
                            so[:].rearrange("p a b -> p (a b)"))
                        yield

                def attn(h, qb, filler):
                    pr = h // 2
                    half = slice(0, 64) if h % 2 == 0 else slice(64, P)
                    ysl = slice(0, 64) if h % 2 == 0 else slice(64, P)
                    ssl = slice(64, P) if h % 2 == 0 else slice(0, 64)
                    nkc = (qb + 1) * 8
                    pv_ps = psPV.tile([P, QB], dt.float32, tag="pv")
                    pend = []          # up to 2 deferred PV emissions

                    def emit_pv(kc, pT_t, qlo):
                        q0 = qlo
                        while q0 < QB:
                            q1 = min((q0 // 512 + 1) * 512, QB)
                            nc.tensor.matmul(pv_ps[:, q0:q1],
                                             V2[:, h, kc, :], pT_t[:, q0:q1],
                                             start=(kc == 0), stop=(kc == nkc - 1),
                                             skip_group_check=True)
                            q0 = q1

                    for kc in range(nkc):
                        qlo = max(0, kc * P - qb * QB)
                        sc_ps = psS.tile([P, QB], dt.float32, tag="sc")
                        q0 = qlo
                        while q0 < QB:
                            q1 = min((q0 // 512 + 1) * 512, QB)
                            nc.tensor.matmul(sc_ps[:, q0:q1],
                                             KT[half, pr, kc * P:(kc + 1) * P],
                                             QT[half, pr, qb * QB + q0:qb * QB + q1],
                                             start=True, stop=True)
                            q0 = q1
                        pT_t = tpt.tile([P, QB], dt.bfloat16, tag="pT")
                        nc.scalar.activation(pT_t[:, qlo:QB], sc_ps[:, qlo:QB],
                                             AF.Exp, scale=EXP_SCALE)
                        if kc * P >= qb * QB:
                            nc.gpsimd.affine_select(
                                out=pT_t[:, qlo:qlo + P], in_=pT_t[:, qlo:qlo + P],
                                compare_op=mybir.AluOpType.is_ge, fill=0.0,
                                base=0, pattern=[[1, P]], channel_multiplier=-1)
                        if len(pend) >= 2:
                            emit_pv(*pend.pop(0))
                        pend.append((kc, pT_t, qlo))
                        if filler:
                            try:
                                next(filler)
                            except StopIteration:
                                filler = None
                    for p_ in pend:
                        emit_pv(*p_)

                    # sums -> SBUF base-0 (approx recip is only exact there),
                    # then scale y rows with a mixed-space cross-partition mult
                    ssum = ta.tile([64, QB], dt.float32, tag="ssum")
                    nc.vector.tensor_copy(ssum[:], pv_ps[ssl, :])
                    rr = ta.tile([64, QB], dt.float32, tag="rr")
                    nc.vector.reciprocal_approx_fast(rr[:], ssum[:])
                    nc.vector.tensor_tensor(yT[ysl, pr, qb * QB:(qb + 1) * QB],
                                            pv_ps[ysl, :], rr[:],
                                            mybir.AluOpType.mult)
                    return filler

                # prologue: QK for pair 0, not overlapped
                for _ in qk_steps(0):
                    pass
                for pr in range(4):
                    filler = qk_steps(pr + 1) if pr < 3 else None
                    for h in (2 * pr, 2 * pr + 1):
                        for qb in range(2):
                            if pr == 3 and h == 7 and qb == 1:
                                filler = proj_steps(0)
                            filler = attn(h, qb, filler)
                    # drain any leftover filler steps
                    while filler is not None:
                        try:
                            next(filler)
                        except StopIteration:
                            filler = None

            # tail: projection of q-block 1
            with tc.tile_pool(name="tso2", bufs=4) as tso2, \
                 tc.tile_pool(name="psP", bufs=4, space="PSUM") as psP:
                for sc in range(8, 16):
                    for oc in range(2):
                        pps = psP.tile([P, 512], dt.float32, tag="pp")
                        for pc in range(4):
                            nc.tensor.matmul(pps[:], yT[:, pc, sc * P:(sc + 1) * P],
                                             wpjT[:, pc, oc * 512:(oc + 1) * 512],
                                             start=(pc == 0), stop=(pc == 3))
                        so = tso2.tile([P, 512], dt.bfloat16, tag="so2")
                        nc.vector.tensor_copy(so[:], pps[:])
                        nc.sync.dma_start(
                            out[sc * P:(sc + 1) * P, oc * 512:(oc + 1) * 512], so[:])

    nc.compile()
    return nc


def prepare_inputs(x, Wqkv, Wproj):
    """Pack per-core inputs. Core c: batch c//2, heads (c%2)*8 .. +8."""
    x = np.asarray(x, dtype=np.float32)
    Wqkv = np.asarray(Wqkv, dtype=np.float32)
    Wproj = np.asarray(Wproj, dtype=np.float32)
    F8 = mybir.dt.np(dt.float8e4)
    Wq = Wqkv[:, :D].reshape(8, P, H, HD)        # [dc, p, head, hd]
    Wk = Wqkv[:, D:2 * D].reshape(8, P, H, HD)
    Wv_ = Wqkv[:, 2 * D:].reshape(8, P, H, HD)
    in_maps = []
    for c in range(8):
        b, g = c // 2, c % 2
        hg = g * NHL
        xtr = np.ascontiguousarray(x[b].T.reshape(8, P, S).transpose(1, 0, 2))
        wqk = np.empty((P, 8, 8, P), dtype=np.float32)
        for ch in range(4):
            wqk[:, :, ch, 0:64] = Wq[:, :, hg + 2 * ch, :].transpose(1, 0, 2)
            wqk[:, :, ch, 64:P] = Wq[:, :, hg + 2 * ch + 1, :].transpose(1, 0, 2)
            wqk[:, :, ch + 4, 0:64] = Wk[:, :, hg + 2 * ch, :].transpose(1, 0, 2)
            wqk[:, :, ch + 4, 64:P] = Wk[:, :, hg + 2 * ch + 1, :].transpose(1, 0, 2)
        wv = Wv_[:, :, hg:hg + NHL, :].reshape(8, P, NHL * HD).transpose(1, 0, 2)
        wpj = np.empty((P, 4, D), dtype=np.float32)
        for pc in range(4):
            wpj[0:64, pc, :] = Wproj[HD * (hg + 2 * pc):HD * (hg + 2 * pc) + HD, :]
            wpj[64:P, pc, :] = Wproj[HD * (hg + 2 * pc + 1):HD * (hg + 2 * pc + 1) + HD, :]
        m = {
            "xt": xtr.astype(BF),
            "wv": np.ascontiguousarray(wv).astype(BF),
            "wpj": wpj.astype(BF),
        }
        if QK_FP8:
            m["xt8"] = xtr.astype(F8)
            m["wqk8"] = np.ascontiguousarray(
                (wqk * W8SCALE).reshape(P, 4, 2, 8, P)).astype(F8)
        else:
            m["wqk"] = wqk.astype(BF)
        in_maps.append(m)
    return in_maps


def combine_outputs(results):
    out = np.empty((B, S, D), dtype=np.float32)
    for b in range(B):
        out[b] = results[2 * b]["out"].astype(np.float32) + \
                 results[2 * b + 1]["out"].astype(np.float32)
    return out


_NC_CACHE = None


def get_nc():
    global _NC_CACHE
    if _NC_CACHE is None:
        _NC_CACHE = build_nc()
    return _NC_CACHE


def kernel(x, Wqkv, Wproj):
    from concourse.bass_utils import run_bass_kernel_spmd
    nc = get_nc()
    in_maps = prepare_inputs(x, Wqkv, Wproj)
    res = run_bass_kernel_spmd(nc, in_maps, core_ids=list(range(8)))
    return combine_outputs(res.results)


if __name__ == "__main__":
    rng = np.random.default_rng(0)
    x = rng.standard_normal((B, S, D), dtype=np.float32)
    Wqkv = (rng.standard_normal((D, 3 * D), dtype=np.float32) / np.sqrt(D)).astype(np.float32)
    Wproj = (rng.standard_normal((D, D), dtype=np.float32) / np.sqrt(D)).astype(np.float32)
    y = kernel(x, Wqkv, Wproj)
    print("ok", y.shape, float(np.abs(y).max()))
